# revision 16
# baseline (speedup 1.0000x reference)
"""Trainium2 Bass kernel for nn_ColorNet: 7x7 box conv s2 -> 3x3 maxpool s2 ->
27 sequential 3x3 box convs (strides [1]*6+[2]+[1]*8+[2]+[1]*11).

Exact decomposition (all filters are separable box filters):
  stage A : Z = B X B^T with B = 256x512 0/1 band (7-tap stride-2, pad 3)
  pool    : 2D 3x3 stride-2 maxpool = max along wo, then max along ro
  tail    : 27-conv chain is linear -> M [32,128] per dim; the
            (1/49)*(1/9)^27 normalization is folded into the tail (1/7 each).

Transpose-light orientation chain (per image; per core 16 images, fp16):
  x     [r 512 (4 chunks on part), c 512]        host-cast fp16, 4-img DMAs
  pass1 V = X_chunk.T @ B^T (lhsT=data)       -> PSUM [c (4 chunks), ro 256]
  crossV PSUM -> SBUF fp16 (ACT/DVE split)    -> Z1 [c part, ro]
  pass2 H = sum_c Z1.T @ B^T (lhsT=Z1)        -> PSUM [ro (2 chunks), wo 256]
  poolH  odd-cols via ACT, maxes on DVE       -> P1 [ro part, w'' 128] bf16
  transp 2x PE transpose (bf16)               -> PSUM [w'' part, ro 256]
  poolV  odd via ACT, maxes on DVE            -> P2 [w'' part, h'' 128] bf16
  tail   g = P2.T @ tw ; yT = g.T @ tw        -> yT = Y^T [32,32] f32
  host   transpose(yT) per image

pass1/pass2 use a 2-colored narrowed-N matmul schedule (banded rhs zones,
start=True exact cover + tiny accumulate fixups) to cut PE streaming 4x.
"""
import numpy as np

N_CORES = 8
N_IMGS = 128
PER_CORE = N_IMGS // N_CORES  # 16

_STRIDES_3x3 = [1] * 6 + [2] + [1] * 8 + [2] + [1] * 11

# narrowed-N schedule: (contributor chunk k, out_start, out_end, start_flag)
# PSUM accumulation (start=False) only works between back-to-back matmuls
# with IDENTICAL output ranges, so: exclusive start=True zones + 3-wide
# boundary stripes as same-range True/False pairs.
_BAND_SCHED = [
    (0, 0, 63, True),
    (0, 63, 66, True), (1, 63, 66, False),
    (1, 66, 127, True),
    (1, 127, 130, True), (2, 127, 130, False),
    (2, 130, 191, True),
    (2, 191, 194, True), (3, 191, 194, False),
    (3, 194, 256, True),
]


def _conv_matrix(n_in, taps, s, p, dtype=np.float64):
    k = len(taps)
    n_out = (n_in + 2 * p - k) // s + 1
    A = np.zeros((n_out, n_in), dtype=dtype)
    for i in range(n_out):
        for j in range(k):
            idx = s * i + j - p
            if 0 <= idx < n_in:
                A[i, idx] = taps[j]
    return A


def _host_consts():
    import ml_dtypes

    B = _conv_matrix(512, [1.0] * 7, 2, 3)  # [256, 512]
    BT = B.T.reshape(4, 128, 256).transpose(1, 0, 2)  # [128, 4, 256]
    wb = np.ascontiguousarray(BT, dtype=np.float16)

    n = 128
    M = np.eye(n)
    for s in _STRIDES_3x3:
        A = _conv_matrix(n, [1 / 3] * 3, s, 1)
        M = A @ M
        n = A.shape[0]
    tw = np.ascontiguousarray(M.T / 7.0, dtype=ml_dtypes.bfloat16)  # [128, 32]
    idb = np.eye(128, dtype=ml_dtypes.bfloat16)

    return {"wb": wb, "tw": tw, "idb": idb}


_NC_CACHE = {}


def _build_nc(reps=1, stage=99):
    """stage: 0=dma, 1=+pass1, 2=+crossV, 3=+pass2, 4=+poolH,
    5=+transpose+poolV, 6=full (+tail)."""
    key = (reps, stage)
    if key in _NC_CACHE:
        return _NC_CACHE[key]
    import contextlib
    import concourse.bass as bass
    import concourse.tile as tile
    from concourse import bacc, mybir

    f32 = mybir.dt.float32
    f16 = mybir.dt.float16
    bf16 = mybir.dt.bfloat16

    nc = bacc.Bacc("TRN2", target_bir_lowering=False, debug=False,
                   num_devices=N_CORES)
    x_d = nc.dram_tensor("x", [PER_CORE, 512, 512], f16,
                         kind="ExternalInput").ap()
    wb_d = nc.dram_tensor("wb", [128, 4, 256], f16, kind="ExternalInput").ap()
    tw_d = nc.dram_tensor("tw", [128, 32], bf16, kind="ExternalInput").ap()
    id_d = nc.dram_tensor("idb", [128, 128], bf16, kind="ExternalInput").ap()
    y_d = nc.dram_tensor("y", [PER_CORE, 1, 32, 32], f32,
                         kind="ExternalOutput").ap()

    with tile.TileContext(nc) as tc:
        with (
            tc.tile_pool(name="consts", bufs=1) as cpool,
            tc.tile_pool(name="x", bufs=2) as xpool,
            tc.tile_pool(name="z1", bufs=3) as z1pool,
            tc.tile_pool(name="ho", bufs=2) as hopool,
            tc.tile_pool(name="p1", bufs=2) as p1pool,
            tc.tile_pool(name="to", bufs=2) as topool,
            tc.tile_pool(name="p2", bufs=2) as p2pool,
            tc.tile_pool(name="g", bufs=2) as gpool,
            tc.tile_pool(name="outs", bufs=1) as opool,
            tc.tile_pool(name="vps", bufs=1, space="PSUM") as vpspool,
            tc.tile_pool(name="hps", bufs=2, space="PSUM") as hpspool,
            tc.tile_pool(name="tps", bufs=1, space="PSUM") as tpspool,
            tc.tile_pool(name="tlps", bufs=1, space="PSUM") as tlpspool,
        ):
            wb = cpool.tile([128, 4, 256], f16, tag="wb")
            nc.sync.dma_start(wb[:], wb_d)
            tw = cpool.tile([128, 32], bf16, tag="tw")
            nc.sync.dma_start(tw[:], tw_d)
            idb = cpool.tile([128, 128], bf16, tag="idb")
            nc.sync.dma_start(idb[:], id_d)
            outs_all = opool.tile([32, PER_CORE * 32], f32, tag="outsall")

            loop_cm = (tc.For_i(0, reps, 1) if reps > 1
                       else contextlib.nullcontext())
            with loop_cm:
              for q in range(PER_CORE // 4):
                # ---- 2 MB DMA: 4 images -> [p, img, rchunk, w] fp16 ----
                xt = xpool.tile([128, 4, 4, 512], f16, tag="xt")
                nc.sync.dma_start(
                    xt[:], x_d[4 * q:4 * q + 4].rearrange(
                        "n (c p) w -> p n c w", p=128))

                def _dummy(n):
                    outt = gpool.tile([32, 32], f32, tag="dummy")
                    nc.vector.tensor_copy(outt[:], xt[0:32, n % 4, 0, 0:32])
                    nc.scalar.copy(outs_all[:, 32 * n:32 * (n + 1)], outt[:])

                for h in range(2):  # image pair within quad
                  ns = [4 * q + 2 * h, 4 * q + 2 * h + 1]
                  if stage < 3:
                    for s in range(2):
                        n = ns[s]
                        if stage < 1:
                            _dummy(n)
                            continue
                        vps = vpspool.tile([128, 4, 256], f32, tag="vps")
                        for i in range(4):
                            for mi, (k, r0, r1, st) in enumerate(_BAND_SCHED):
                                nc.tensor.matmul(
                                    vps[:, i, r0:r1],
                                    xt[:, n % 4, k, 128 * i:128 * (i + 1)],
                                    wb[:, k, r0:r1],
                                    start=st,
                                    stop=(mi == len(_BAND_SCHED) - 1),
                                    skip_group_check=True)
                        if stage < 2:
                            _dummy(n)
                            continue
                        z1 = z1pool.tile([128, 4, 256], f16, tag="z1")
                        vf = vps[:].rearrange("p i r -> p (i r)")
                        z1f = z1[:].rearrange("p i r -> p (i r)")
                        nc.scalar.copy(z1f[:, 0:640], vf[:, 0:640])
                        nc.vector.tensor_copy(z1f[:, 640:1024], vf[:, 640:1024])
                        _dummy(n)
                    continue

                  h2 = hpspool.tile([128, 2, 2, 256], f32, tag="h2")
                  for s in range(2):
                    n = ns[s]
                    # ---- pass 1 ----
                    vps = vpspool.tile([128, 4, 256], f32, tag="vps")
                    for i in range(4):
                        for mi, (k, r0, r1, st) in enumerate(_BAND_SCHED):
                            nc.tensor.matmul(
                                vps[:, i, r0:r1],
                                xt[:, n % 4, k, 128 * i:128 * (i + 1)],
                                wb[:, k, r0:r1],
                                start=st, stop=(mi == len(_BAND_SCHED) - 1),
                                skip_group_check=True)
                    # ---- crossV: PSUM f32 -> SBUF fp16 ----
                    z1 = z1pool.tile([128, 4, 256], f16, tag="z1")
                    vf = vps[:].rearrange("p i r -> p (i r)")
                    z1f = z1[:].rearrange("p i r -> p (i r)")
                    nc.scalar.copy(z1f[:, 0:640], vf[:, 0:640])
                    nc.vector.tensor_copy(z1f[:, 640:1024], vf[:, 640:1024])
                    # ---- pass 2: H = sum_cc Z1_cc.T @ B^T ----
                    for j in range(2):
                        for mi, (cc, w0, w1, st) in enumerate(_BAND_SCHED):
                            nc.tensor.matmul(
                                h2[:, s, j, w0:w1],
                                z1[:, cc, 128 * j:128 * (j + 1)],
                                wb[:, cc, w0:w1],
                                start=st, stop=(mi == len(_BAND_SCHED) - 1),
                                skip_group_check=True)

                  if stage < 4:
                    for s in range(2):
                        _dummy(ns[s])
                    continue

                  # ---- pool-H (pair-batched): odd cols ACT, maxes DVE ----
                  h4 = h2[:].rearrange("p s j (w t) -> p s j w t", t=2)
                  ho = hopool.tile([128, 2, 2, 128], bf16, tag="ho")
                  nc.scalar.copy(ho[:], h4[:, :, :, :, 1])
                  p1 = p1pool.tile([128, 2, 2, 128], bf16, tag="p1")
                  nc.vector.tensor_max(p1[:], h4[:, :, :, :, 0], ho[:])
                  nc.vector.tensor_max(p1[:, :, :, 1:128], p1[:, :, :, 1:128],
                                       ho[:, :, :, 0:127])

                  if stage < 5:
                    for s in range(2):
                        _dummy(ns[s])
                    continue

                  # ---- transposes: [ro-chunk, w''] -> [w'', ro-chunk] ----
                  tp = tpspool.tile([128, 2, 2, 128], bf16, tag="tp")
                  for s in range(2):
                      for j in range(2):
                          nc.tensor.transpose(tp[:, s, j, :], p1[:, s, j, :],
                                              idb[:])
                  # ---- pool-V (pair-batched) ----
                  t4 = tp[:].rearrange("p s j (i t) -> p s (j i) t", t=2)
                  to = topool.tile([128, 2, 128], bf16, tag="to")
                  nc.scalar.copy(to[:], t4[:, :, :, 1])
                  p2 = p2pool.tile([128, 2, 128], bf16, tag="p2")
                  nc.vector.tensor_max(p2[:], t4[:, :, :, 0], to[:])
                  nc.vector.tensor_max(p2[:, :, 1:128], p2[:, :, 1:128],
                                       to[:, :, 0:127])

                  if stage < 6:
                    for s in range(2):
                        _dummy(ns[s])
                    continue

                  # ---- tail: g = P2.T @ tw ; yT = g.T @ tw ----
                  tl = tlpspool.tile([128, 2, 64], f32, tag="tl")
                  for s in range(2):
                      nc.tensor.matmul(tl[:, s, 0:32], p2[:, s, :], tw[:],
                                       start=True, stop=True)
                  gb = gpool.tile([128, 2, 32], bf16, tag="gb")
                  nc.scalar.copy(gb[:], tl[:, :, 0:32])
                  for s in range(2):
                      nc.tensor.matmul(tl[0:32, s, 32:64], gb[:, s, :], tw[:],
                                       start=True, stop=True)
                  nc.scalar.copy(
                      outs_all[:, 32 * ns[0]:32 * (ns[1] + 1)],
                      tl[0:32, :, 32:64])

              nc.sync.dma_start(
                  y_d[:, 0].rearrange("n h w -> h n w"),
                  outs_all[:].rearrange("h (n w) -> h n w", w=32))

    nc.compile()
    _NC_CACHE[key] = nc
    return nc


def make_in_maps(x, filter1=None, filter2=None):
    x = np.asarray(x)
    assert x.shape == (N_IMGS, 1, 512, 512)
    x16 = np.ascontiguousarray(x.reshape(N_IMGS, 512, 512).astype(np.float16))
    consts = _host_consts()
    in_maps = []
    for c in range(N_CORES):
        m = {"x": x16[c * PER_CORE:(c + 1) * PER_CORE]}
        m.update(consts)
        in_maps.append(m)
    return in_maps


def build_nc(reps=1):
    return _build_nc(reps=reps)


def kernel(x, filter1, filter2):
    from concourse.bass_utils import run_bass_kernel_spmd

    in_maps = make_in_maps(x, filter1, filter2)
    nc = _build_nc()
    res = run_bass_kernel_spmd(nc, in_maps, list(range(N_CORES)))
    y = np.concatenate([res.results[c]["y"] for c in range(N_CORES)], axis=0)
    # device emits Y^T per image (tail orientation); undo on host
    return np.ascontiguousarray(y.transpose(0, 1, 3, 2)).astype(np.float32)


# revision 21
# speedup vs baseline: 1.4397x; 1.4397x over previous
"""Trainium2 Bass kernel for nn_ColorNet: 7x7 box conv s2 -> 3x3 maxpool s2 ->
27 sequential 3x3 box convs (strides [1]*6+[2]+[1]*8+[2]+[1]*11).

Exact decomposition (all filters are separable box filters):
  stage A : Z = B X B^T with B = 256x512 0/1 band (7-tap stride-2, pad 3)
  pool    : 2D 3x3 stride-2 maxpool = max along wo, then max along ro
  tail    : 27-conv chain is linear -> M [32,128] per dim; the
            (1/49)*(1/9)^27 normalization is folded into the tail (1/7 each).

Transpose-light orientation chain (per image; per core 16 images, fp16):
  x     [r 512 (4 chunks on part), c 512]        host-cast fp16, 4-img DMAs
  pass1 V = X_chunk.T @ B^T (lhsT=data)       -> PSUM [c (4 chunks), ro 256]
  crossV PSUM -> SBUF fp16 (ACT/DVE split)    -> Z1 [c part, ro]
  pass2 H = sum_c Z1.T @ B^T (lhsT=Z1)        -> PSUM [ro (2 chunks), wo 256]
  poolH  odd-cols via ACT, maxes on DVE       -> P1 [ro part, w'' 128] bf16
  transp 2x PE transpose (bf16)               -> PSUM [w'' part, ro 256]
  poolV  odd via ACT, maxes on DVE            -> P2 [w'' part, h'' 128] bf16
  tail   g = P2.T @ tw ; yT = g.T @ tw        -> yT = Y^T [32,32] f32
  host   transpose(yT) per image

pass1/pass2 use a 2-colored narrowed-N matmul schedule (banded rhs zones,
start=True exact cover + tiny accumulate fixups) to cut PE streaming 4x.
"""
import numpy as np

N_CORES = 8
N_IMGS = 128
PER_CORE = N_IMGS // N_CORES  # 16

_STRIDES_3x3 = [1] * 6 + [2] + [1] * 8 + [2] + [1] * 11

# narrowed-N schedule: (contributor chunk k, out_start, out_end, start_flag)
# PSUM accumulation (start=False) only works between back-to-back matmuls
# with IDENTICAL output ranges, so: exclusive start=True zones + 3-wide
# boundary stripes as same-range True/False pairs.
_BAND_SCHED = [
    (0, 0, 63, True),
    (0, 63, 66, True), (1, 63, 66, False),
    (1, 66, 127, True),
    (1, 127, 130, True), (2, 127, 130, False),
    (2, 130, 191, True),
    (2, 191, 194, True), (3, 191, 194, False),
    (3, 194, 256, True),
]


def _conv_matrix(n_in, taps, s, p, dtype=np.float64):
    k = len(taps)
    n_out = (n_in + 2 * p - k) // s + 1
    A = np.zeros((n_out, n_in), dtype=dtype)
    for i in range(n_out):
        for j in range(k):
            idx = s * i + j - p
            if 0 <= idx < n_in:
                A[i, idx] = taps[j]
    return A


def _host_consts():
    import ml_dtypes

    B = _conv_matrix(512, [1.0] * 7, 2, 3)  # [256, 512]
    BT = B.T.reshape(4, 128, 256).transpose(1, 0, 2)  # [128, 4, 256]
    wb = np.ascontiguousarray(BT, dtype=np.float16)

    n = 128
    M = np.eye(n)
    for s in _STRIDES_3x3:
        A = _conv_matrix(n, [1 / 3] * 3, s, 1)
        M = A @ M
        n = A.shape[0]
    tw = np.ascontiguousarray(M.T / 7.0, dtype=ml_dtypes.bfloat16)  # [128, 32]
    idb = np.eye(128, dtype=ml_dtypes.bfloat16)

    return {"wb": wb, "tw": tw, "idb": idb}


_NC_CACHE = {}


def _build_nc(reps=1, stage=99):
    """stage: 0=dma, 1=+pass1, 2=+crossV, 3=+pass2, 4=+poolH,
    5=+transpose+poolV, 6=full (+tail)."""
    key = (reps, stage)
    if key in _NC_CACHE:
        return _NC_CACHE[key]
    import contextlib
    import concourse.bass as bass
    import concourse.tile as tile
    from concourse import bacc, mybir

    f32 = mybir.dt.float32
    f16 = mybir.dt.float16
    bf16 = mybir.dt.bfloat16

    nc = bacc.Bacc("TRN2", target_bir_lowering=False, debug=False,
                   num_devices=N_CORES)
    # host pre-tiled: [p, n, rchunk, w] so each partition reads sequential HBM
    x_d = nc.dram_tensor("x", [128, PER_CORE, 4, 512], f16,
                         kind="ExternalInput").ap()
    wb_d = nc.dram_tensor("wb", [128, 4, 256], f16, kind="ExternalInput").ap()
    tw_d = nc.dram_tensor("tw", [128, 32], bf16, kind="ExternalInput").ap()
    id_d = nc.dram_tensor("idb", [128, 128], bf16, kind="ExternalInput").ap()
    y_d = nc.dram_tensor("y", [PER_CORE, 1, 32, 32], f32,
                         kind="ExternalOutput").ap()

    with tile.TileContext(nc) as tc:
        with (
            tc.tile_pool(name="consts", bufs=1) as cpool,
            tc.tile_pool(name="x", bufs=2) as xpool,
            tc.tile_pool(name="z1", bufs=3) as z1pool,
            tc.tile_pool(name="ho", bufs=2) as hopool,
            tc.tile_pool(name="p1", bufs=2) as p1pool,
            tc.tile_pool(name="to", bufs=2) as topool,
            tc.tile_pool(name="p2", bufs=2) as p2pool,
            tc.tile_pool(name="g", bufs=2) as gpool,
            tc.tile_pool(name="outs", bufs=1) as opool,
            tc.tile_pool(name="vps", bufs=2, space="PSUM") as vpspool,
            tc.tile_pool(name="hps", bufs=2, space="PSUM") as hpspool,
            tc.tile_pool(name="tps", bufs=2, space="PSUM") as tpspool,
            tc.tile_pool(name="tlps", bufs=2, space="PSUM") as tlpspool,
        ):
            wb = cpool.tile([128, 4, 256], f16, tag="wb")
            nc.sync.dma_start(wb[:], wb_d)
            tw = cpool.tile([128, 32], bf16, tag="tw")
            nc.sync.dma_start(tw[:], tw_d)
            idb = cpool.tile([128, 128], bf16, tag="idb")
            nc.sync.dma_start(idb[:], id_d)
            outs_all = opool.tile([32, PER_CORE * 32], f32, tag="outsall")

            loop_cm = (tc.For_i(0, reps, 1) if reps > 1
                       else contextlib.nullcontext())
            with loop_cm:
              for q in range(PER_CORE // 4):
                # ---- 2 MB DMA: 4 images -> [p, img, rchunk, w] fp16 ----
                xt = xpool.tile([128, 4, 4, 512], f16, tag="xt")
                nc.sync.dma_start(xt[:], x_d[:, 4 * q:4 * q + 4])

                def _dummy(n):
                    outt = gpool.tile([32, 32], f32, tag="dummy")
                    nc.vector.tensor_copy(outt[:], xt[0:32, n % 4, 0, 0:32])
                    nc.scalar.copy(outs_all[:, 32 * n:32 * (n + 1)], outt[:])

                for h in range(2):  # image pair within quad
                  ns = [4 * q + 2 * h, 4 * q + 2 * h + 1]
                  p1 = p1pool.tile([128, 2, 2, 128], bf16, tag="p1")
                  for s in range(2):
                    n = ns[s]
                    if stage < 1:
                        _dummy(n)
                        continue
                    # ---- pass 1 + crossV in halves (1 PSUM bank each) ----
                    z1 = z1pool.tile([128, 4, 256], f16, tag="z1")
                    for hf in range(2):
                        vps = vpspool.tile([128, 2, 256], f32, tag="vps")
                        for i2 in range(2):
                            i = 2 * hf + i2
                            for mi, (k, r0, r1, st) in enumerate(_BAND_SCHED):
                                nc.tensor.matmul(
                                    vps[:, i2, r0:r1],
                                    xt[:, n % 4, k, 128 * i:128 * (i + 1)],
                                    wb[:, k, r0:r1],
                                    start=st,
                                    stop=(mi == len(_BAND_SCHED) - 1),
                                    skip_group_check=True)
                        if stage < 2:
                            continue
                        vf = vps[:].rearrange("p i r -> p (i r)")
                        z1f = z1[:, 2 * hf:2 * hf + 2].rearrange(
                            "p i r -> p (i r)")
                        nc.scalar.copy(z1f[:, 0:320], vf[:, 0:320])
                        nc.vector.tensor_copy(z1f[:, 320:512], vf[:, 320:512])
                    if stage < 3:
                        _dummy(n)
                        continue
                    # ---- pass 2: H = sum_cc Z1_cc.T @ B^T ----
                    h2 = hpspool.tile([128, 2, 256], f32, tag="h2")
                    for j in range(2):
                        for mi, (cc, w0, w1, st) in enumerate(_BAND_SCHED):
                            nc.tensor.matmul(
                                h2[:, j, w0:w1],
                                z1[:, cc, 128 * j:128 * (j + 1)],
                                wb[:, cc, w0:w1],
                                start=st, stop=(mi == len(_BAND_SCHED) - 1),
                                skip_group_check=True)
                    if stage < 4:
                        _dummy(n)
                        continue
                    # ---- pool-H: odd cols via ACT, maxes on DVE ----
                    h4 = h2[:].rearrange("p j (w t) -> p j w t", t=2)
                    ho = hopool.tile([128, 2, 128], bf16, tag="ho")
                    nc.scalar.copy(ho[:], h4[:, :, :, 1])
                    nc.vector.tensor_max(p1[:, s], h4[:, :, :, 0], ho[:])
                    nc.vector.tensor_max(p1[:, s, :, 1:128],
                                         p1[:, s, :, 1:128],
                                         ho[:, :, 0:127])

                  if stage < 4:
                    continue

                  if stage < 5:
                    for s in range(2):
                        _dummy(ns[s])
                    continue

                  # ---- transposes: [ro-chunk, w''] -> [w'', ro-chunk] ----
                  tp = tpspool.tile([128, 2, 2, 128], bf16, tag="tp")
                  for s in range(2):
                      for j in range(2):
                          nc.tensor.transpose(tp[:, s, j, :], p1[:, s, j, :],
                                              idb[:])
                  # ---- pool-V (pair-batched) ----
                  t4 = tp[:].rearrange("p s j (i t) -> p s (j i) t", t=2)
                  to = topool.tile([128, 2, 128], bf16, tag="to")
                  nc.scalar.copy(to[:], t4[:, :, :, 1])
                  p2 = p2pool.tile([128, 2, 128], bf16, tag="p2")
                  nc.vector.tensor_max(p2[:], t4[:, :, :, 0], to[:])
                  nc.vector.tensor_max(p2[:, :, 1:128], p2[:, :, 1:128],
                                       to[:, :, 0:127])

                  if stage < 6:
                    for s in range(2):
                        _dummy(ns[s])
                    continue

                  # ---- tail: g = P2.T @ tw ; yT = g.T @ tw ----
                  tl = tlpspool.tile([128, 2, 64], f32, tag="tl")
                  for s in range(2):
                      nc.tensor.matmul(tl[:, s, 0:32], p2[:, s, :], tw[:],
                                       start=True, stop=True)
                  gb = gpool.tile([128, 2, 32], bf16, tag="gb")
                  nc.scalar.copy(gb[:], tl[:, :, 0:32])
                  for s in range(2):
                      nc.tensor.matmul(tl[0:32, s, 32:64], gb[:, s, :], tw[:],
                                       start=True, stop=True)
                  nc.scalar.copy(
                      outs_all[:, 32 * ns[0]:32 * (ns[1] + 1)],
                      tl[0:32, :, 32:64])

              nc.sync.dma_start(
                  y_d[:, 0].rearrange("n h w -> h n w"),
                  outs_all[:].rearrange("h (n w) -> h n w", w=32))

    nc.compile()
    _NC_CACHE[key] = nc
    return nc


def make_in_maps(x, filter1=None, filter2=None):
    x = np.asarray(x)
    assert x.shape == (N_IMGS, 1, 512, 512)
    # [n, 512, 512] -> per-core [p, n, rchunk, w] fp16 (sequential HBM reads)
    x16 = x.reshape(N_IMGS, 4, 128, 512).astype(np.float16)
    consts = _host_consts()
    in_maps = []
    for c in range(N_CORES):
        xc = x16[c * PER_CORE:(c + 1) * PER_CORE]  # [16, 4, 128, 512]
        m = {"x": np.ascontiguousarray(xc.transpose(2, 0, 1, 3))}
        m.update(consts)
        in_maps.append(m)
    return in_maps


def build_nc(reps=1):
    return _build_nc(reps=reps)


def kernel(x, filter1, filter2):
    from concourse.bass_utils import run_bass_kernel_spmd

    in_maps = make_in_maps(x, filter1, filter2)
    nc = _build_nc()
    res = run_bass_kernel_spmd(nc, in_maps, list(range(N_CORES)))
    y = np.concatenate([res.results[c]["y"] for c in range(N_CORES)], axis=0)
    # device emits Y^T per image (tail orientation); undo on host
    return np.ascontiguousarray(y.transpose(0, 1, 3, 2)).astype(np.float32)
